# revision 10
# baseline (speedup 1.0000x reference)
"""Trainium2 Bass kernel for nn_DiffOp_8830452760922 (GNN message passing).

Strategy
--------
Each of the 4 message-passing layers applies a *single linear* message
function before a scatter-mean, so the per-edge matmul collapses
algebraically:

    mean_e W_m [x_src; x_dst] + b_m
      = W_l (mean_e x_src) + W_r x_dst + b_m          (cnt > 0 everywhere)

and the update fuses further on the host:

    x' = (Wu@Wl) Smean + (Wu@Wr + Ws) x + (Wu@bm + bs + bu)

so the device work per layer is: a row-gather of x_src for every edge
(bf16 tables), a one-hot matmul scatter-mean on the PE (the 1/cnt weight is
folded into the one-hot values), two dense [d x dout] matmuls per 128-node
tile, and a softplus.  The scatter one-hots are built on-chip by the vector
engine -- oh = (iota == dstpos) * inv -- from two compact [128, TOTCH]
tables, instead of streaming 128x128 prebuilt one-hots from DRAM (which
would roughly double the kernel's HBM traffic).

Sharding: destination nodes are sharded across the 8 cores (6272 nodes
each). Each core processes exactly the edges landing in its node range and
holds a replicated bf16 gather table, rebuilt per layer via two AllGather
collectives (the table is split into A/B halves so each stays under the
int16 gather-index limit; the boundary table C is computed redundantly on
every core).
"""

import numpy as np
import ml_dtypes

import concourse.bass as bass
import concourse.bacc as bacc
import concourse.tile as tile
import concourse.mybir as mybir
from concourse.bass_utils import run_bass_kernel_spmd
from concourse.masks import make_identity

BF16 = mybir.dt.bfloat16
F32 = mybir.dt.float32
I16 = mybir.dt.int16

N_INT, N_BOUND = 50000, 2000
NCORE = 8
NLOC = 6272                 # interior nodes per core (padded: 8*6272 = 50176)
TILES = NLOC // 128         # 49
A_LOC, B_LOC = 4096, NLOC - 4096
NA, NB, NC_ = NCORE * A_LOC, NCORE * B_LOC, 2048   # 32768, 17408, 2048
D_IN, D_EMB = 64, 128
DINS = [64, 128, 128, 128]
DOUTS = [128, 128, 128, 64]
SG = 4                      # dst tiles per gather super-group
INST_CHUNKS = 48            # max chunks per dma_gather instruction
GROUP_ROWS = [NA, NB, NC_]

_cache = {}


# --------------------------------------------------------------------------
# host preprocessing
# --------------------------------------------------------------------------
def _chunk_layout(K):
    """K: [TILES][3] chunk counts. Returns (chunk_meta, chunk_of, gather_insts,
    supergroups)."""
    sgs = [list(range(s, min(s + SG, TILES))) for s in range(0, TILES, SG)]
    chunk_meta = []
    chunk_of = {}
    gather_insts = []   # (grp, c0, nchunks)
    for sg in sgs:
        for g in range(3):
            c0 = len(chunk_meta)
            for t in sg:
                chunk_of[(t, g)] = (len(chunk_meta), K[t][g])
                chunk_meta += [(t, g)] * K[t][g]
            n = len(chunk_meta) - c0
            while n > 0:
                take = min(n, INST_CHUNKS)
                gather_insts.append((g, c0, take))
                c0 += take
                n -= take
    return chunk_meta, chunk_of, gather_insts, sgs


def _host_prep(inputs):
    t = float(np.asarray(inputs["t"]))
    ts = np.asarray(inputs["timestamps"], np.float32)
    bvv = np.asarray(inputs["boundary_values"], np.float32)  # [T,1,2000,64]
    i = int(np.searchsorted(ts, t))
    il, ir = max(i - 1, 0), min(i, ts.shape[0] - 1)
    if ts[il] == ts[ir]:
        bv_t = bvv[il][0]
    else:
        bv_t = (bvv[il] + (t - ts[il]) * (bvv[ir] - bvv[il]) / (ts[ir] - ts[il]))[0]
    bv_t = np.asarray(bv_t, np.float32)                      # [2000, 64]

    x0 = np.asarray(inputs["x_int"][0], np.float32)          # [50000, 64]
    ei = np.asarray(inputs["edge_index_int"])
    eb = np.asarray(inputs["edge_index_bound"])
    src = np.concatenate([ei[0], eb[0]]).astype(np.int64)
    dst = np.concatenate([ei[1], eb[1]]).astype(np.int64)
    E = src.shape[0]

    cnt = np.bincount(dst, minlength=N_INT).astype(np.float32)
    assert (cnt > 0).all(), "zero in-degree node: fused-weight path invalid"
    inv = 1.0 / cnt                                          # [N_INT]

    # table mapping for sources
    interior = src < N_INT
    c_src = src // NLOC
    l_src = src - c_src * NLOC
    grp = np.where(interior, np.where(l_src < A_LOC, 0, 1), 2).astype(np.int64)
    tix = np.where(
        interior,
        np.where(l_src < A_LOC, c_src * A_LOC + l_src,
                 c_src * B_LOC + (l_src - A_LOC)),
        src - N_INT,
    ).astype(np.int64)
    assert tix.max() < 32768

    core_of = dst // NLOC
    lloc = dst - core_of * NLOC
    tile_of = lloc // 128
    pos_of = lloc % 128

    key = (core_of * TILES + tile_of) * 3 + grp
    nkey = NCORE * TILES * 3
    counts = np.bincount(key, minlength=nkey).reshape(NCORE, TILES, 3)
    K = np.ceil(counts / 128).astype(np.int64).max(axis=0)   # [TILES, 3]
    chunk_meta, chunk_of, gather_insts, sgs = _chunk_layout(K)
    TOTCH = len(chunk_meta)
    SLOTS = TOTCH * 128

    # slot index for each edge
    order = np.argsort(key, kind="stable")
    run_start = np.zeros(nkey, np.int64)
    run_start[1:] = np.cumsum(np.bincount(key, minlength=nkey))[:-1]
    rank = np.arange(E, dtype=np.int64) - run_start[key[order]]
    # chunk base (in slots) for each (t, g)
    cb = np.zeros((TILES, 3), np.int64)
    for (tt, gg), (c0, kk) in chunk_of.items():
        cb[tt, gg] = c0 * 128
    slot = cb[tile_of[order], grp[order]] + rank

    idx_all = np.zeros((NCORE, SLOTS), np.int16)
    dstp_all = np.full((NCORE, SLOTS), 255.0, np.float32)
    invd_all = np.zeros((NCORE, SLOTS), np.float32)
    co = core_of[order]
    idx_all[co, slot] = tix[order].astype(np.int16)
    dstp_all[co, slot] = pos_of[order].astype(np.float32)
    invd_all[co, slot] = inv[dst[order]]

    # device layouts
    idx_lay = idx_all.reshape(NCORE, SLOTS // 16, 16).transpose(0, 2, 1)  # [8,16,W]
    idx_lay = np.ascontiguousarray(np.tile(idx_lay, (1, 8, 1)))           # [8,128,W]
    # compact scatter metadata: the 128x128 one-hot for chunk c is built
    # on-chip as (iota == dpos[:, c]) * invw[:, c]
    dpos_lay = [np.ascontiguousarray(dstp_all[c].reshape(TOTCH, 128).T
                                     .astype(ml_dtypes.bfloat16)) for c in range(NCORE)]
    invw_lay = [np.ascontiguousarray(invd_all[c].reshape(TOTCH, 128).T
                                     .astype(ml_dtypes.bfloat16)) for c in range(NCORE)]
    iota_np = np.ascontiguousarray(
        np.tile(np.arange(128, dtype=np.float32), (128, 1)).astype(ml_dtypes.bfloat16))

    # layer-1 gather tables (bf16, rows padded to 128 channels)
    xpad = np.zeros((NCORE * NLOC, D_IN), np.float32)
    xpad[:N_INT] = x0
    t1a = np.zeros((NA, 128), ml_dtypes.bfloat16)
    t1b = np.zeros((NB, 128), ml_dtypes.bfloat16)
    t1c = np.zeros((NC_, 128), ml_dtypes.bfloat16)
    for c in range(NCORE):
        s = c * NLOC
        t1a[c * A_LOC:(c + 1) * A_LOC, :D_IN] = xpad[s:s + A_LOC]
        t1b[c * B_LOC:(c + 1) * B_LOC, :D_IN] = xpad[s + A_LOC:s + NLOC]
    t1c[:N_BOUND, :D_IN] = bv_t

    xT0 = np.zeros((NCORE, D_IN, NLOC), np.float32)
    for c in range(NCORE):
        xT0[c] = xpad[c * NLOC:(c + 1) * NLOC].T
    bvT0 = np.zeros((D_IN, NC_), np.float32)
    bvT0[:, :N_BOUND] = bv_t.T

    weights = {}
    for li in range(1, 5):
        d = DINS[li - 1]
        Wm = np.asarray(inputs[f"Wm{li}"], np.float64)
        bm = np.asarray(inputs[f"bm{li}"], np.float64)
        Ws = np.asarray(inputs[f"Ws{li}"], np.float64)
        bs = np.asarray(inputs[f"bs{li}"], np.float64)
        Wu = np.asarray(inputs[f"Wu{li}"], np.float64)
        bu = np.asarray(inputs[f"bu{li}"], np.float64)
        Wl, Wr = Wm[:, :d], Wm[:, d:]
        weights[f"waT{li}"] = np.ascontiguousarray((Wu @ Wl).T.astype(np.float32))
        weights[f"wbT{li}"] = np.ascontiguousarray((Wu @ Wr + Ws).T.astype(np.float32))
        weights[f"bc{li}"] = (Wu @ bm + bs + bu).astype(np.float32)[:, None]
        if li < 4:
            weights[f"wsT{li}"] = np.ascontiguousarray(Ws.T.astype(np.float32))
            weights[f"bs{li}"] = bs.astype(np.float32)[:, None]
    weights["bc4row"] = np.ascontiguousarray(weights["bc4"].T)  # [1, 64]

    shared = dict(t1a=np.asarray(t1a), t1b=np.asarray(t1b), t1c=np.asarray(t1c),
                  bvT0=bvT0, iota=iota_np, **weights)
    per_core = []
    for c in range(NCORE):
        per_core.append(dict(shared, idx=idx_lay[c], dpos=dpos_lay[c],
                             invw=invw_lay[c], xT0=xT0[c]))
    struct = dict(K=tuple(map(tuple, K)), TOTCH=TOTCH, SLOTS=SLOTS)
    return per_core, struct


# --------------------------------------------------------------------------
# device program
# --------------------------------------------------------------------------
def _build_nc(struct):
    K = [list(r) for r in struct["K"]]
    TOTCH = struct["TOTCH"]
    chunk_meta, chunk_of, gather_insts, sgs = _chunk_layout(K)
    W = TOTCH * 8  # idx columns

    nc = bacc.Bacc("TRN2", target_bir_lowering=False, debug=False,
                   num_devices=NCORE, num_swdge_queues=4)

    t1 = [nc.dram_tensor(n, [r, 128], BF16, kind="ExternalInput")
          for n, r in (("t1a", NA), ("t1b", NB), ("t1c", NC_))]
    idx_d = nc.dram_tensor("idx", [128, W], I16, kind="ExternalInput")
    dpos_d = nc.dram_tensor("dpos", [128, TOTCH], BF16, kind="ExternalInput")
    invw_d = nc.dram_tensor("invw", [128, TOTCH], BF16, kind="ExternalInput")
    iota_d = nc.dram_tensor("iota", [128, 128], BF16, kind="ExternalInput")
    xT0_d = nc.dram_tensor("xT0", [D_IN, NLOC], F32, kind="ExternalInput")
    bvT0_d = nc.dram_tensor("bvT0", [D_IN, NC_], F32, kind="ExternalInput")
    wd = {}
    for li in range(1, 5):
        d, do = DINS[li - 1], DOUTS[li - 1]
        wd[f"waT{li}"] = nc.dram_tensor(f"waT{li}", [d, do], F32, kind="ExternalInput")
        wd[f"wbT{li}"] = nc.dram_tensor(f"wbT{li}", [d, do], F32, kind="ExternalInput")
        wd[f"bc{li}"] = nc.dram_tensor(f"bc{li}", [do, 1], F32, kind="ExternalInput")
        if li < 4:
            wd[f"wsT{li}"] = nc.dram_tensor(f"wsT{li}", [d, 128], F32, kind="ExternalInput")
            wd[f"bs{li}"] = nc.dram_tensor(f"bs{li}", [128, 1], F32, kind="ExternalInput")
    wd["bc4row"] = nc.dram_tensor("bc4row", [1, 64], F32, kind="ExternalInput")
    out_d = nc.dram_tensor("out", [NLOC, DOUTS[3]], F32, kind="ExternalOutput")

    with tile.TileContext(nc) as tc:
        with (
            tc.tile_pool(name="const", bufs=1) as const,
            tc.tile_pool(name="xb", bufs=1) as xb,
            tc.tile_pool(name="gst", bufs=2) as gpool,
            tc.tile_pool(name="ohb", bufs=6) as ohbp,
            tc.tile_pool(name="stp", bufs=4) as stp,
            tc.tile_pool(name="misc", bufs=3) as misc,
            tc.tile_pool(name="tsg", bufs=2) as tsg,
            tc.tile_pool(name="pacc", bufs=5, space="PSUM") as pacc_p,
            tc.tile_pool(name="pup", bufs=2, space="PSUM") as pup,
            tc.tile_pool(name="ptr", bufs=1, space="PSUM") as ptrp,
            tc.tile_pool(name="dram", bufs=1, space="DRAM") as dram,
        ):
            # ---- resident constants
            idx_sb = const.tile([128, W], I16)
            nc.sync.dma_start(idx_sb[:], idx_d[:])
            dpos_sb = const.tile([128, TOTCH], BF16)
            nc.sync.dma_start(dpos_sb[:], dpos_d[:])
            invw_sb = const.tile([128, TOTCH], BF16)
            nc.sync.dma_start(invw_sb[:], invw_d[:])
            iota_sb = const.tile([128, 128], BF16)
            nc.sync.dma_start(iota_sb[:], iota_d[:])
            idbf = const.tile([128, 128], BF16)
            make_identity(nc, idbf[:])
            idf32 = const.tile([64, 64], F32)
            make_identity(nc, idf32[:])
            ones_sb = const.tile([1, 128], F32)
            nc.vector.memset(ones_sb[:], 1.0)
            wsb = {}
            for name, hd in wd.items():
                w = const.tile(list(hd.shape), F32, name=f"w_{name}")
                nc.sync.dma_start(w[:], hd[:])
                wsb[name] = w

            # ---- x / bv feature buffers (channel-major fp32)
            x_sb = [xb.tile([D_IN, NLOC], F32, name="x0buf"),
                    xb.tile([128, NLOC], F32, name="xAbuf"),
                    xb.tile([128, NLOC], F32, name="xBbuf")]
            nc.sync.dma_start(x_sb[0][:], xT0_d[:])
            bv_sb = [xb.tile([D_IN, NC_], F32, name="bv0buf"),
                     xb.tile([128, NC_], F32, name="bvAbuf"),
                     xb.tile([128, NC_], F32, name="bvBbuf")]
            nc.sync.dma_start(bv_sb[0][:], bvT0_d[:])

            # ---- per-boundary DRAM tables
            tabs = {1: [t1[0][:], t1[1][:], t1[2][:]]}
            cc_in = {}
            for li in (2, 3, 4):
                ta = dram.tile([NA, 128], BF16, name=f"TA{li}", addr_space="Shared")
                tb = dram.tile([NB, 128], BF16, name=f"TB{li}", addr_space="Shared")
                tc_ = dram.tile([NC_, 128], BF16, name=f"TC{li}")
                tabs[li] = [ta, tb, tc_]
                cc_in[li] = [dram.tile([A_LOC, 128], BF16, name=f"ccA{li}"),
                             dram.tile([B_LOC, 128], BF16, name=f"ccB{li}")]

            for li in range(1, 5):
                d, do = DINS[li - 1], DOUTS[li - 1]
                xT = x_sb[0] if li == 1 else x_sb[1 + (li % 2)]
                xTn = x_sb[1 + ((li + 1) % 2)]      # layers 1..3 write here
                bvT = bv_sb[0] if li == 1 else bv_sb[1 + (li % 2)]
                bvTn = bv_sb[1 + ((li + 1) % 2)]
                tabA, tabB, tabC = tabs[li]
                tabsrc = [tabA, tabB, tabC]

                # gather instructions for this layer, keyed by chunk range
                gtiles = []   # (c0, n, gather tile)
                def flush_sg(sg_insts):
                    for (g, c0, n) in sg_insts:
                        gt = gpool.tile([128, INST_CHUNKS, 128], BF16,
                                        name=f"g{li}", tag="gst")
                        nc.gpsimd.dma_gather(
                            out_ap=gt[:, :n, :],
                            in_ap=tabsrc[g][:, :],
                            idxs_ap=idx_sb[:, c0 * 8:(c0 + n) * 8],
                            num_idxs=n * 128,
                            num_idxs_reg=n * 128,
                            elem_size=128,
                            single_packet=False,
                            queue_num=len(gtiles) % 4,
                        )
                        gtiles.append((c0, n, gt))

                def g_slice(cg):
                    for (c0, n, gt) in reversed(gtiles):
                        if c0 <= cg < c0 + n:
                            return gt[:, cg - c0, :d]
                    raise KeyError(cg)

                inst_i = 0
                for sgi, sg in enumerate(sgs):
                    # issue this supergroup's gathers
                    mine = []
                    while inst_i < len(gather_insts):
                        g, c0, n = gather_insts[inst_i]
                        t0 = chunk_meta[c0][0]
                        if t0 in sg:
                            mine.append((g, c0, n))
                            inst_i += 1
                        else:
                            break
                    flush_sg(mine)

                    nsg = len(sg)
                    tstage = tsg.tile([128, SG, 128], BF16, name=f"ts{li}", tag="ts") \
                        if li < 4 else None
                    ostage = tsg.tile([128, SG, 64], F32, name=f"os{li}", tag="os") \
                        if li == 4 else None
                    for k, t in enumerate(sg):
                        chunks = []
                        for g in range(3):
                            c0, kk = chunk_of[(t, g)]
                            chunks += list(range(c0, c0 + kk))
                        pacc = pacc_p.tile([d, 128], F32, name=f"pa{li}", tag="pacc")
                        for ci, cg in enumerate(chunks):
                            glhs = g_slice(cg)
                            goh = ohbp.tile([128, 128], BF16, name=f"oh{li}",
                                            tag="ohb")
                            nc.vector.tensor_scalar(
                                out=goh[:], in0=iota_sb[:],
                                scalar1=dpos_sb[:, cg:cg + 1],
                                scalar2=invw_sb[:, cg:cg + 1],
                                op0=mybir.AluOpType.is_equal,
                                op1=mybir.AluOpType.mult,
                            )
                            nc.tensor.matmul(
                                out=pacc[:], lhsT=glhs, rhs=goh[:],
                                start=(ci == 0), stop=(ci == len(chunks) - 1),
                            )
                        stile = stp.tile([d, 128], F32, name=f"st{li}", tag="st")
                        nc.scalar.activation(
                            out=stile[:], in_=pacc[:],
                            func=mybir.ActivationFunctionType.Copy)
                        p3 = pup.tile([do, 128], F32, name=f"p3{li}", tag="p3")
                        nc.tensor.matmul(p3[:], lhsT=wsb[f"waT{li}"][:], rhs=stile[:],
                                         start=True, stop=False)
                        nc.tensor.matmul(p3[:], lhsT=wsb[f"wbT{li}"][:],
                                         rhs=xT[:d, t * 128:(t + 1) * 128],
                                         start=False, stop=(li == 4 and False) or li < 4)
                        if li < 4:
                            # softplus(z) = ln(1 + exp(z)); Softplus itself is
                            # not in any loadable ACT table, Exp+Ln are.
                            ex = misc.tile([128, 128], F32, name=f"ex{li}", tag="ex")
                            nc.scalar.activation(
                                out=ex[:], in_=p3[:],
                                func=mybir.ActivationFunctionType.Exp,
                                bias=wsb[f"bc{li}"][:])
                            nc.scalar.activation(
                                out=xTn[:, t * 128:(t + 1) * 128], in_=ex[:],
                                func=mybir.ActivationFunctionType.Ln, bias=1.0)
                            xbf = misc.tile([128, 128], BF16, name=f"xbf{li}", tag="xbf")
                            nc.scalar.activation(
                                out=xbf[:], in_=ex[:],
                                func=mybir.ActivationFunctionType.Ln, bias=1.0)
                            tp = ptrp.tile([128, 128], BF16, name=f"tp{li}", tag="tp")
                            nc.tensor.transpose(tp[:], xbf[:], idbf[:])
                            nc.vector.tensor_copy(out=tstage[:, k, :], in_=tp[:])
                        else:
                            nc.tensor.matmul(p3[:], lhsT=wsb["bc4row"][:],
                                             rhs=ones_sb[:], start=False, stop=True)
                            ob = misc.tile([64, 128], F32, name="ob4", tag="xbf")
                            nc.scalar.activation(
                                out=ob[:], in_=p3[:],
                                func=mybir.ActivationFunctionType.Copy)
                            tp4 = ptrp.tile([128, 64], F32, name="tp4", tag="tp")
                            nc.tensor.transpose(tp4[:], ob[:], idf32[:])
                            nc.vector.tensor_copy(out=ostage[:, k, :], in_=tp4[:])

                    # flush staging to DRAM
                    r0 = sg[0] * 128
                    nrow = nsg * 128
                    if li < 4:
                        dst_tab = cc_in[li + 1][0] if sg[0] < 32 else cc_in[li + 1][1]
                        roff = r0 if sg[0] < 32 else r0 - A_LOC
                        ov = dst_tab[roff:roff + nrow, :].rearrange(
                            "(k p) c -> p k c", p=128)
                        nc.sync.dma_start(out=ov, in_=tstage[:, :nsg, :])
                    else:
                        ov = out_d[r0:r0 + nrow, :].rearrange("(k p) c -> p k c", p=128)
                        nc.sync.dma_start(out=ov, in_=ostage[:, :nsg, :])

                if li < 4:
                    # boundary-node update (replicated on every core)
                    for s in range(4):
                        pb = pup.tile([128, 512], F32, name=f"pb{li}", tag="p3")
                        nc.tensor.matmul(pb[:], lhsT=wsb[f"wsT{li}"][:],
                                         rhs=bvT[:d, s * 512:(s + 1) * 512],
                                         start=True, stop=True)
                        bex = misc.tile([128, 512], F32, name=f"bex{li}", tag="bex")
                        nc.scalar.activation(
                            out=bex[:], in_=pb[:],
                            func=mybir.ActivationFunctionType.Exp,
                            bias=wsb[f"bs{li}"][:])
                        if li < 3:
                            nc.scalar.activation(
                                out=bvTn[:, s * 512:(s + 1) * 512], in_=bex[:],
                                func=mybir.ActivationFunctionType.Ln, bias=1.0)
                        bvbf = misc.tile([128, 512], BF16, name=f"bvbf{li}", tag="bvbf")
                        nc.scalar.activation(
                            out=bvbf[:], in_=bex[:],
                            func=mybir.ActivationFunctionType.Ln, bias=1.0)
                        tstage = tsg.tile([128, SG, 128], BF16, name=f"tsb{li}", tag="ts")
                        for k in range(4):
                            tp = ptrp.tile([128, 128], BF16, name=f"tpb{li}", tag="tp")
                            nc.tensor.transpose(tp[:], bvbf[:, k * 128:(k + 1) * 128],
                                                idbf[:])
                            nc.vector.tensor_copy(out=tstage[:, k, :], in_=tp[:])
                        ov = tabs[li + 1][2][s * 512:(s + 1) * 512, :].rearrange(
                            "(k p) c -> p k c", p=128)
                        nc.sync.dma_start(out=ov, in_=tstage[:, :, :])

                    # all-gather the interior table halves
                    nc.gpsimd.collective_compute(
                        "AllGather", mybir.AluOpType.bypass,
                        replica_groups=[list(range(NCORE))],
                        ins=[cc_in[li + 1][0][:]], outs=[tabs[li + 1][0][:]])
                    nc.gpsimd.collective_compute(
                        "AllGather", mybir.AluOpType.bypass,
                        replica_groups=[list(range(NCORE))],
                        ins=[cc_in[li + 1][1][:]], outs=[tabs[li + 1][1][:]])

    nc.compile()
    return nc


# --------------------------------------------------------------------------
TRACE = False          # test harness can flip this to capture an NTFF profile
TRACE_CORES = [0]      # which cores to profile (fewer = faster test turnaround)
last_results = None    # BassKernelResults of the most recent kernel() call


def kernel(**inputs) -> np.ndarray:
    global last_results
    per_core, struct = _host_prep(inputs)
    key = struct["K"]
    if key not in _cache:
        _cache[key] = _build_nc(struct)
    nc = _cache[key]
    kw = {}
    if TRACE:
        kw = dict(trace=True, trace_cores=list(TRACE_CORES))
    res = run_bass_kernel_spmd(nc, per_core, core_ids=list(range(NCORE)), **kw)
    last_results = res
    out = np.concatenate([res.results[c]["out"] for c in range(NCORE)], axis=0)
    return out[None, :N_INT, :].astype(np.float32)



# revision 20
# speedup vs baseline: 1.1889x; 1.1889x over previous
"""Trainium2 Bass kernel for nn_DiffOp_8830452760922 (GNN message passing).

Strategy
--------
Each of the 4 message-passing layers applies a *single linear* message
function before a scatter-mean, so the per-edge matmul collapses
algebraically:

    mean_e W_m [x_src; x_dst] + b_m
      = W_l (mean_e x_src) + W_r x_dst + b_m          (cnt > 0 everywhere)

and the update fuses further on the host:

    x' = (Wu@Wl) Smean + (Wu@Wr + Ws) x + (Wu@bm + bs + bu)

so the device work per layer is: a row-gather of x_src for every edge
(bf16 tables), a one-hot matmul scatter-mean on the PE (the 1/cnt weight is
folded into the one-hot values), two dense [d x dout] matmuls per 128-node
tile, and a softplus.  The scatter one-hots are built on-chip by the vector
engine -- oh = (iota == dstpos) * inv -- from two compact [128, TOTCH]
tables, instead of streaming 128x128 prebuilt one-hots from DRAM (which
would roughly double the kernel's HBM traffic).

Sharding: destination nodes are sharded across the 8 cores (6272 nodes
each). Each core processes exactly the edges landing in its node range and
holds a replicated bf16 gather table, rebuilt per layer via two AllGather
collectives (the table is split into A/B halves so each stays under the
int16 gather-index limit; the boundary table C is computed redundantly on
every core).
"""

import numpy as np
import ml_dtypes

import concourse.bass as bass
import concourse.bacc as bacc
import concourse.tile as tile
import concourse.mybir as mybir
from concourse.bass_utils import run_bass_kernel_spmd
from concourse.masks import make_identity

BF16 = mybir.dt.bfloat16
F32 = mybir.dt.float32
I16 = mybir.dt.int16

N_INT, N_BOUND = 50000, 2000
NCORE = 8
NLOC = 6272                 # interior nodes per core (padded: 8*6272 = 50176)
TILES = NLOC // 128         # 49
A_LOC, B_LOC = 4096, NLOC - 4096
NA, NB, NC_ = NCORE * A_LOC, NCORE * B_LOC, 2048   # 32768, 17408, 2048
D_IN, D_EMB = 64, 128
DINS = [64, 128, 128, 128]
DOUTS = [128, 128, 128, 64]
SG = 4                      # dst tiles per gather super-group
INST_CHUNKS = 16            # max chunks per dma_gather instruction
GROUP_ROWS = [NA, NB, NC_]

_cache = {}


# --------------------------------------------------------------------------
# host preprocessing
# --------------------------------------------------------------------------
def _chunk_layout(K):
    """K: [TILES][3] chunk counts. Returns (chunk_meta, chunk_of, gather_insts,
    supergroups)."""
    sgs = [list(range(s, min(s + SG, TILES))) for s in range(0, TILES, SG)]
    chunk_meta = []
    chunk_of = {}
    gather_insts = []   # (grp, c0, nchunks)
    for sg in sgs:
        for g in (0, 2, 1):   # B last: its table's AllGather lands latest
            c0 = len(chunk_meta)
            for t in sg:
                chunk_of[(t, g)] = (len(chunk_meta), K[t][g])
                chunk_meta += [(t, g)] * K[t][g]
            n = len(chunk_meta) - c0
            while n > 0:
                take = min(n, INST_CHUNKS)
                gather_insts.append((g, c0, take))
                c0 += take
                n -= take
    return chunk_meta, chunk_of, gather_insts, sgs


def _host_prep(inputs):
    t = float(np.asarray(inputs["t"]))
    ts = np.asarray(inputs["timestamps"], np.float32)
    bvv = np.asarray(inputs["boundary_values"], np.float32)  # [T,1,2000,64]
    i = int(np.searchsorted(ts, t))
    il, ir = max(i - 1, 0), min(i, ts.shape[0] - 1)
    if ts[il] == ts[ir]:
        bv_t = bvv[il][0]
    else:
        bv_t = (bvv[il] + (t - ts[il]) * (bvv[ir] - bvv[il]) / (ts[ir] - ts[il]))[0]
    bv_t = np.asarray(bv_t, np.float32)                      # [2000, 64]

    x0 = np.asarray(inputs["x_int"][0], np.float32)          # [50000, 64]
    ei = np.asarray(inputs["edge_index_int"])
    eb = np.asarray(inputs["edge_index_bound"])
    src = np.concatenate([ei[0], eb[0]]).astype(np.int64)
    dst = np.concatenate([ei[1], eb[1]]).astype(np.int64)
    E = src.shape[0]

    cnt = np.bincount(dst, minlength=N_INT).astype(np.float32)
    assert (cnt > 0).all(), "zero in-degree node: fused-weight path invalid"
    inv = 1.0 / cnt                                          # [N_INT]

    # table mapping for sources
    interior = src < N_INT
    c_src = src // NLOC
    l_src = src - c_src * NLOC
    grp = np.where(interior, np.where(l_src < A_LOC, 0, 1), 2).astype(np.int64)
    tix = np.where(
        interior,
        np.where(l_src < A_LOC, c_src * A_LOC + l_src,
                 c_src * B_LOC + (l_src - A_LOC)),
        src - N_INT,
    ).astype(np.int64)
    assert tix.max() < 32768

    core_of = dst // NLOC
    lloc = dst - core_of * NLOC
    tile_of = lloc // 128
    pos_of = lloc % 128

    key = (core_of * TILES + tile_of) * 3 + grp
    nkey = NCORE * TILES * 3
    counts = np.bincount(key, minlength=nkey).reshape(NCORE, TILES, 3)
    K = np.ceil(counts / 128).astype(np.int64).max(axis=0)   # [TILES, 3]
    chunk_meta, chunk_of, gather_insts, sgs = _chunk_layout(K)
    TOTCH = len(chunk_meta)
    SLOTS = TOTCH * 128

    # slot index for each edge
    order = np.argsort(key, kind="stable")
    run_start = np.zeros(nkey, np.int64)
    run_start[1:] = np.cumsum(np.bincount(key, minlength=nkey))[:-1]
    rank = np.arange(E, dtype=np.int64) - run_start[key[order]]
    # chunk base (in slots) for each (t, g)
    cb = np.zeros((TILES, 3), np.int64)
    for (tt, gg), (c0, kk) in chunk_of.items():
        cb[tt, gg] = c0 * 128
    slot = cb[tile_of[order], grp[order]] + rank

    idx_all = np.zeros((NCORE, SLOTS), np.int16)
    dstp_all = np.full((NCORE, SLOTS), 255.0, np.float32)
    invd_all = np.zeros((NCORE, SLOTS), np.float32)
    co = core_of[order]
    idx_all[co, slot] = tix[order].astype(np.int16)
    dstp_all[co, slot] = pos_of[order].astype(np.float32)
    invd_all[co, slot] = inv[dst[order]]

    # device layouts
    idx_lay = idx_all.reshape(NCORE, SLOTS // 16, 16).transpose(0, 2, 1)  # [8,16,W]
    idx_lay = np.ascontiguousarray(np.tile(idx_lay, (1, 8, 1)))           # [8,128,W]
    # compact scatter metadata: the 128x128 one-hot for chunk c is built
    # on-chip as (iota == dpos[:, c]) * invw[:, c]
    dpos_lay = [np.ascontiguousarray(dstp_all[c].reshape(TOTCH, 128).T)
                for c in range(NCORE)]
    invw_lay = [np.ascontiguousarray(
        invd_all[c].reshape(TOTCH, 128).T
        .astype(ml_dtypes.bfloat16).astype(np.float32)) for c in range(NCORE)]
    iota_np = np.ascontiguousarray(
        np.tile(np.arange(128, dtype=np.float32), (128, 1)).astype(ml_dtypes.bfloat16))

    # layer-1 gather tables (bf16, rows padded to 128 channels)
    xpad = np.zeros((NCORE * NLOC, D_IN), np.float32)
    xpad[:N_INT] = x0
    t1a = np.zeros((NA, 128), ml_dtypes.bfloat16)
    t1b = np.zeros((NB, 128), ml_dtypes.bfloat16)
    t1c = np.zeros((NC_, 128), ml_dtypes.bfloat16)
    for c in range(NCORE):
        s = c * NLOC
        t1a[c * A_LOC:(c + 1) * A_LOC, :D_IN] = xpad[s:s + A_LOC]
        t1b[c * B_LOC:(c + 1) * B_LOC, :D_IN] = xpad[s + A_LOC:s + NLOC]
    t1c[:N_BOUND, :D_IN] = bv_t

    xT0 = np.zeros((NCORE, D_IN, NLOC), np.float32)
    for c in range(NCORE):
        xT0[c] = xpad[c * NLOC:(c + 1) * NLOC].T
    bvT0 = np.zeros((D_IN, NC_), np.float32)
    bvT0[:, :N_BOUND] = bv_t.T

    weights = {}
    for li in range(1, 5):
        d = DINS[li - 1]
        Wm = np.asarray(inputs[f"Wm{li}"], np.float64)
        bm = np.asarray(inputs[f"bm{li}"], np.float64)
        Ws = np.asarray(inputs[f"Ws{li}"], np.float64)
        bs = np.asarray(inputs[f"bs{li}"], np.float64)
        Wu = np.asarray(inputs[f"Wu{li}"], np.float64)
        bu = np.asarray(inputs[f"bu{li}"], np.float64)
        Wl, Wr = Wm[:, :d], Wm[:, d:]
        weights[f"waT{li}"] = np.ascontiguousarray((Wu @ Wl).T.astype(np.float32))
        weights[f"wbT{li}"] = np.ascontiguousarray((Wu @ Wr + Ws).T.astype(np.float32))
        weights[f"bc{li}"] = (Wu @ bm + bs + bu).astype(np.float32)[:, None]
        if li < 4:
            weights[f"wsT{li}"] = np.ascontiguousarray(Ws.T.astype(np.float32))
            weights[f"bs{li}"] = bs.astype(np.float32)[:, None]
    weights["bc4row"] = np.ascontiguousarray(weights["bc4"].T)  # [1, 64]

    shared = dict(t1a=np.asarray(t1a), t1b=np.asarray(t1b), t1c=np.asarray(t1c),
                  bvT0=bvT0, iota=iota_np, **weights)
    per_core = []
    for c in range(NCORE):
        per_core.append(dict(shared, idx=idx_lay[c], dpos=dpos_lay[c],
                             invw=invw_lay[c], xT0=xT0[c]))
    struct = dict(K=tuple(map(tuple, K)), TOTCH=TOTCH, SLOTS=SLOTS)
    return per_core, struct


# --------------------------------------------------------------------------
# device program
# --------------------------------------------------------------------------
def _build_nc(struct):
    K = [list(r) for r in struct["K"]]
    TOTCH = struct["TOTCH"]
    chunk_meta, chunk_of, gather_insts, sgs = _chunk_layout(K)
    W = TOTCH * 8  # idx columns

    nc = bacc.Bacc("TRN2", target_bir_lowering=False, debug=False,
                   num_devices=NCORE, num_swdge_queues=4)

    t1 = [nc.dram_tensor(n, [r, 128], BF16, kind="ExternalInput")
          for n, r in (("t1a", NA), ("t1b", NB), ("t1c", NC_))]
    idx_d = nc.dram_tensor("idx", [128, W], I16, kind="ExternalInput")
    dpos_d = nc.dram_tensor("dpos", [128, TOTCH], F32, kind="ExternalInput")
    invw_d = nc.dram_tensor("invw", [128, TOTCH], F32, kind="ExternalInput")
    iota_d = nc.dram_tensor("iota", [128, 128], BF16, kind="ExternalInput")
    xT0_d = nc.dram_tensor("xT0", [D_IN, NLOC], F32, kind="ExternalInput")
    bvT0_d = nc.dram_tensor("bvT0", [D_IN, NC_], F32, kind="ExternalInput")
    wd = {}
    for li in range(1, 5):
        d, do = DINS[li - 1], DOUTS[li - 1]
        wd[f"waT{li}"] = nc.dram_tensor(f"waT{li}", [d, do], F32, kind="ExternalInput")
        wd[f"wbT{li}"] = nc.dram_tensor(f"wbT{li}", [d, do], F32, kind="ExternalInput")
        wd[f"bc{li}"] = nc.dram_tensor(f"bc{li}", [do, 1], F32, kind="ExternalInput")
        if li < 4:
            wd[f"wsT{li}"] = nc.dram_tensor(f"wsT{li}", [d, 128], F32, kind="ExternalInput")
            wd[f"bs{li}"] = nc.dram_tensor(f"bs{li}", [128, 1], F32, kind="ExternalInput")
    wd["bc4row"] = nc.dram_tensor("bc4row", [1, 64], F32, kind="ExternalInput")
    out_d = nc.dram_tensor("out", [NLOC, DOUTS[3]], F32, kind="ExternalOutput")

    with tile.TileContext(nc) as tc:
        with (
            tc.tile_pool(name="const", bufs=1) as const,
            tc.tile_pool(name="xb", bufs=1) as xb,
            tc.tile_pool(name="gst", bufs=10) as gpool,
            tc.tile_pool(name="ohb", bufs=6) as ohbp,
            tc.tile_pool(name="stp", bufs=4) as stp,
            tc.tile_pool(name="misc", bufs=3) as misc,
            tc.tile_pool(name="tsg", bufs=2) as tsg,
            tc.tile_pool(name="pacc", bufs=5, space="PSUM") as pacc_p,
            tc.tile_pool(name="pup", bufs=2, space="PSUM") as pup,
            tc.tile_pool(name="ptr", bufs=1, space="PSUM") as ptrp,
            tc.tile_pool(name="dram", bufs=1, space="DRAM") as dram,
        ):
            # ---- resident constants
            idx_sb = const.tile([128, W], I16)
            nc.sync.dma_start(idx_sb[:], idx_d[:])
            dpos_sb = const.tile([128, TOTCH], F32)
            nc.sync.dma_start(dpos_sb[:], dpos_d[:])
            invw_sb = const.tile([128, TOTCH], F32)
            nc.sync.dma_start(invw_sb[:], invw_d[:])
            iota_sb = const.tile([128, 128], BF16)
            nc.sync.dma_start(iota_sb[:], iota_d[:])
            idbf = const.tile([128, 128], BF16)
            make_identity(nc, idbf[:])
            idf32 = const.tile([64, 64], F32)
            make_identity(nc, idf32[:])
            ones_sb = const.tile([1, 128], F32)
            nc.vector.memset(ones_sb[:], 1.0)
            wsb = {}
            for name, hd in wd.items():
                w = const.tile(list(hd.shape), F32, name=f"w_{name}")
                nc.sync.dma_start(w[:], hd[:])
                wsb[name] = w

            # ---- x / bv feature buffers (channel-major fp32)
            x_sb = [xb.tile([D_IN, NLOC], F32, name="x0buf"),
                    xb.tile([128, NLOC], F32, name="xAbuf"),
                    xb.tile([128, NLOC], F32, name="xBbuf")]
            nc.sync.dma_start(x_sb[0][:], xT0_d[:])
            bv_sb = [xb.tile([D_IN, NC_], F32, name="bv0buf"),
                     xb.tile([128, NC_], F32, name="bvAbuf"),
                     xb.tile([128, NC_], F32, name="bvBbuf")]
            nc.sync.dma_start(bv_sb[0][:], bvT0_d[:])

            # ---- per-boundary DRAM tables
            tabs = {1: [t1[0][:], t1[1][:], t1[2][:]]}
            cc_in = {}
            for li in (2, 3, 4):
                ta = dram.tile([NA, 128], BF16, name=f"TA{li}", addr_space="Shared")
                tb = dram.tile([NB, 128], BF16, name=f"TB{li}", addr_space="Shared")
                tc_ = dram.tile([NC_, 128], BF16, name=f"TC{li}")
                tabs[li] = [ta, tb, tc_]
                cc_in[li] = [dram.tile([A_LOC, 128], BF16, name=f"ccA{li}"),
                             dram.tile([B_LOC, 128], BF16, name=f"ccB{li}")]

            for li in range(1, 5):
                d, do = DINS[li - 1], DOUTS[li - 1]
                xT = x_sb[0] if li == 1 else x_sb[1 + (li % 2)]
                xTn = x_sb[1 + ((li + 1) % 2)]      # layers 1..3 write here
                bvT = bv_sb[0] if li == 1 else bv_sb[1 + (li % 2)]
                bvTn = bv_sb[1 + ((li + 1) % 2)]
                tabA, tabB, tabC = tabs[li]
                tabsrc = [tabA, tabB, tabC]

                # gather instructions for this layer, keyed by chunk range
                gtiles = []   # (c0, n, gather tile)
                def flush_sg(sg_insts):
                    for (g, c0, n) in sg_insts:
                        gt = gpool.tile([128, INST_CHUNKS, 128], BF16,
                                        name=f"g{li}", tag="gst")
                        nc.gpsimd.dma_gather(
                            out_ap=gt[:, :n, :],
                            in_ap=tabsrc[g][:, :],
                            idxs_ap=idx_sb[:, c0 * 8:(c0 + n) * 8],
                            num_idxs=n * 128,
                            num_idxs_reg=n * 128,
                            elem_size=128,
                            single_packet=False,
                            queue_num=len(gtiles) % 4,
                        )
                        gtiles.append((c0, n, gt))

                def g_slice(cg):
                    for (c0, n, gt) in reversed(gtiles):
                        if c0 <= cg < c0 + n:
                            return gt[:, cg - c0, :d]
                    raise KeyError(cg)

                if li < 4:
                    # boundary-node update (replicated on every core); done at
                    # layer start so the C table write + bv' compute overlap
                    # the first supergroups' gathers.
                    for s in range(4):
                        pb = pup.tile([128, 512], F32, name=f"pb{li}", tag="p3")
                        nc.tensor.matmul(pb[:], lhsT=wsb[f"wsT{li}"][:],
                                         rhs=bvT[:d, s * 512:(s + 1) * 512],
                                         start=True, stop=True)
                        bex = misc.tile([128, 512], F32, name=f"bex{li}", tag="bex")
                        nc.scalar.activation(
                            out=bex[:], in_=pb[:],
                            func=mybir.ActivationFunctionType.Exp,
                            bias=wsb[f"bs{li}"][:])
                        if li < 3:
                            nc.scalar.activation(
                                out=bvTn[:, s * 512:(s + 1) * 512], in_=bex[:],
                                func=mybir.ActivationFunctionType.Ln, bias=1.0)
                        bvbf = misc.tile([128, 512], BF16, name=f"bvbf{li}", tag="bvbf")
                        nc.scalar.activation(
                            out=bvbf[:], in_=bex[:],
                            func=mybir.ActivationFunctionType.Ln, bias=1.0)
                        tstage = tsg.tile([128, SG, 128], BF16, name=f"tsb{li}", tag="ts")
                        for k in range(4):
                            tp = ptrp.tile([128, 128], BF16, name=f"tpb{li}", tag="tp")
                            nc.tensor.transpose(tp[:], bvbf[:, k * 128:(k + 1) * 128],
                                                idbf[:])
                            nc.vector.tensor_copy(out=tstage[:, k, :], in_=tp[:])
                        ov = tabs[li + 1][2][s * 512:(s + 1) * 512, :].rearrange(
                            "(k p) c -> p k c", p=128)
                        nc.sync.dma_start(out=ov, in_=tstage[:, :, :])

                inst_i = 0
                for sgi, sg in enumerate(sgs):
                    # issue this supergroup's gathers
                    mine = []
                    while inst_i < len(gather_insts):
                        g, c0, n = gather_insts[inst_i]
                        t0 = chunk_meta[c0][0]
                        if t0 in sg:
                            mine.append((g, c0, n))
                            inst_i += 1
                        else:
                            break
                    flush_sg(mine)
                    if li < 4 and sgi == 9:
                        # A half was fully staged ~2 supergroups ago (tile 31,
                        # sgi 7); all-gather it now -- late enough that the
                        # Pool engine won't stall waiting on the staging DMAs,
                        # early enough to overlap the remaining supergroups.
                        nc.gpsimd.collective_compute(
                            "AllGather", mybir.AluOpType.bypass,
                            replica_groups=[list(range(NCORE))],
                            ins=[cc_in[li + 1][0][:]],
                            outs=[tabs[li + 1][0][:]])

                    nsg = len(sg)
                    tstage = tsg.tile([128, SG, 128], BF16, name=f"ts{li}", tag="ts") \
                        if li < 4 else None
                    ostage = tsg.tile([128, SG, 64], F32, name=f"os{li}", tag="os") \
                        if li == 4 else None
                    for k, t in enumerate(sg):
                        chunks = []
                        for g in (0, 2, 1):   # consume in gather-arrival order
                            c0, kk = chunk_of[(t, g)]
                            chunks += list(range(c0, c0 + kk))
                        pacc = pacc_p.tile([d, 128], F32, name=f"pa{li}", tag="pacc")
                        for ci, cg in enumerate(chunks):
                            glhs = g_slice(cg)
                            goh = ohbp.tile([128, 128], BF16, name=f"oh{li}",
                                            tag="ohb")
                            nc.vector.tensor_scalar(
                                out=goh[:], in0=iota_sb[:],
                                scalar1=dpos_sb[:, cg:cg + 1],
                                scalar2=invw_sb[:, cg:cg + 1],
                                op0=mybir.AluOpType.is_equal,
                                op1=mybir.AluOpType.mult,
                            )
                            nc.tensor.matmul(
                                out=pacc[:], lhsT=glhs, rhs=goh[:],
                                start=(ci == 0), stop=(ci == len(chunks) - 1),
                            )
                        stile = stp.tile([d, 128], F32, name=f"st{li}", tag="st")
                        nc.vector.tensor_copy(out=stile[:], in_=pacc[:])
                        p3 = pup.tile([do, 128], F32, name=f"p3{li}", tag="p3")
                        nc.tensor.matmul(p3[:], lhsT=wsb[f"waT{li}"][:], rhs=stile[:],
                                         start=True, stop=False)
                        nc.tensor.matmul(p3[:], lhsT=wsb[f"wbT{li}"][:],
                                         rhs=xT[:d, t * 128:(t + 1) * 128],
                                         start=False, stop=(li == 4 and False) or li < 4)
                        if li < 4:
                            # softplus(z) = ln(1 + exp(z)); Softplus itself is
                            # not in any loadable ACT table, Exp+Ln are.
                            ex = misc.tile([128, 128], F32, name=f"ex{li}", tag="ex")
                            nc.scalar.activation(
                                out=ex[:], in_=p3[:],
                                func=mybir.ActivationFunctionType.Exp,
                                bias=wsb[f"bc{li}"][:])
                            nc.scalar.activation(
                                out=xTn[:, t * 128:(t + 1) * 128], in_=ex[:],
                                func=mybir.ActivationFunctionType.Ln, bias=1.0)
                            xbf = misc.tile([128, 128], BF16, name=f"xbf{li}", tag="xbf")
                            nc.scalar.activation(
                                out=xbf[:], in_=ex[:],
                                func=mybir.ActivationFunctionType.Ln, bias=1.0)
                            tp = ptrp.tile([128, 128], BF16, name=f"tp{li}", tag="tp")
                            nc.tensor.transpose(tp[:], xbf[:], idbf[:])
                            nc.vector.tensor_copy(out=tstage[:, k, :], in_=tp[:])
                        else:
                            nc.tensor.matmul(p3[:], lhsT=wsb["bc4row"][:],
                                             rhs=ones_sb[:], start=False, stop=True)
                            ob = misc.tile([64, 128], F32, name="ob4", tag="xbf")
                            nc.scalar.activation(
                                out=ob[:], in_=p3[:],
                                func=mybir.ActivationFunctionType.Copy)
                            tp4 = ptrp.tile([128, 64], F32, name="tp4", tag="tp")
                            nc.tensor.transpose(tp4[:], ob[:], idf32[:])
                            nc.vector.tensor_copy(out=ostage[:, k, :], in_=tp4[:])

                    # flush staging to DRAM
                    r0 = sg[0] * 128
                    nrow = nsg * 128
                    if li < 4:
                        dst_tab = cc_in[li + 1][0] if sg[0] < 32 else cc_in[li + 1][1]
                        roff = r0 if sg[0] < 32 else r0 - A_LOC
                        ov = dst_tab[roff:roff + nrow, :].rearrange(
                            "(k p) c -> p k c", p=128)
                        nc.sync.dma_start(out=ov, in_=tstage[:, :nsg, :])
                    else:
                        ov = out_d[r0:r0 + nrow, :].rearrange("(k p) c -> p k c", p=128)
                        nc.sync.dma_start(out=ov, in_=ostage[:, :nsg, :])

                if li < 4:
                    # all-gather the (smaller) B half at layer end
                    nc.gpsimd.collective_compute(
                        "AllGather", mybir.AluOpType.bypass,
                        replica_groups=[list(range(NCORE))],
                        ins=[cc_in[li + 1][1][:]], outs=[tabs[li + 1][1][:]])

    nc.compile()
    return nc


# --------------------------------------------------------------------------
TRACE = False          # test harness can flip this to capture an NTFF profile
TRACE_CORES = [0]      # which cores to profile (fewer = faster test turnaround)
last_results = None    # BassKernelResults of the most recent kernel() call


def kernel(**inputs) -> np.ndarray:
    global last_results
    per_core, struct = _host_prep(inputs)
    key = struct["K"]
    if key not in _cache:
        _cache[key] = _build_nc(struct)
    nc = _cache[key]
    kw = {}
    if TRACE:
        kw = dict(trace=True, trace_cores=list(TRACE_CORES))
    res = run_bass_kernel_spmd(nc, per_core, core_ids=list(range(NCORE)), **kw)
    last_results = res
    out = np.concatenate([res.results[c]["out"] for c in range(NCORE)], axis=0)
    return out[None, :N_INT, :].astype(np.float32)



# revision 23
# speedup vs baseline: 1.7196x; 1.4464x over previous
"""Trainium2 Bass kernel for nn_DiffOp_8830452760922 (GNN message passing).

Strategy
--------
Each of the 4 message-passing layers applies a *single linear* message
function before a scatter-mean, so the per-edge matmul collapses
algebraically:

    mean_e W_m [x_src; x_dst] + b_m
      = W_l (mean_e x_src) + W_r x_dst + b_m          (cnt > 0 everywhere)

and the update fuses further on the host:

    x' = (Wu@Wl) Smean + (Wu@Wr + Ws) x + (Wu@bm + bs + bu)

so the device work per layer is: a row-gather of x_src for every edge
(bf16 tables), a one-hot matmul scatter-mean on the PE (the 1/cnt weight is
folded into the one-hot values), two dense [d x dout] matmuls per 128-node
tile, and a softplus.  The scatter one-hots are built on-chip by the vector
engine -- oh = (iota == dstpos) * inv -- from two compact [128, TOTCH]
tables, instead of streaming 128x128 prebuilt one-hots from DRAM (which
would roughly double the kernel's HBM traffic).

Sharding: destination nodes are sharded across the 8 cores (6272 nodes
each). Each core processes exactly the edges landing in its node range and
holds a replicated bf16 gather table, rebuilt per layer via two AllGather
collectives (the table is split into A/B halves so each stays under the
int16 gather-index limit; the boundary table C is computed redundantly on
every core).
"""

import numpy as np
import ml_dtypes

import concourse.bass as bass
import concourse.bacc as bacc
import concourse.tile as tile
import concourse.mybir as mybir
from concourse.bass_utils import run_bass_kernel_spmd
from concourse.masks import make_identity

BF16 = mybir.dt.bfloat16
F32 = mybir.dt.float32
I16 = mybir.dt.int16

N_INT, N_BOUND = 50000, 2000
NCORE = 8
NLOC = 6272                 # interior nodes per core (padded: 8*6272 = 50176)
TILES = NLOC // 128         # 49
A_LOC, B_LOC = 4096, NLOC - 4096
NA, NB, NC_ = NCORE * A_LOC, NCORE * B_LOC, 2048   # 32768, 17408, 2048
D_IN, D_EMB = 64, 128
DINS = [64, 128, 128, 128]
DOUTS = [128, 128, 128, 64]
SG = 4                      # dst tiles per gather super-group
INST_CHUNKS = 16            # max chunks per dma_gather instruction
GROUP_ROWS = [NA, NB, NC_]

_cache = {}


# --------------------------------------------------------------------------
# host preprocessing
# --------------------------------------------------------------------------
def _chunk_layout(K):
    """K: [TILES][3] chunk counts. Returns (chunk_meta, chunk_of, gather_insts,
    supergroups)."""
    sgs = [list(range(s, min(s + SG, TILES))) for s in range(0, TILES, SG)]
    chunk_meta = []
    chunk_of = {}
    gather_insts = []   # (grp, c0, nchunks)
    for sg in sgs:
        for g in (0, 2, 1):   # B last: its table's AllGather lands latest
            c0 = len(chunk_meta)
            for t in sg:
                chunk_of[(t, g)] = (len(chunk_meta), K[t][g])
                chunk_meta += [(t, g)] * K[t][g]
            n = len(chunk_meta) - c0
            while n > 0:
                take = min(n, INST_CHUNKS)
                gather_insts.append((g, c0, take))
                c0 += take
                n -= take
    return chunk_meta, chunk_of, gather_insts, sgs


def _host_prep(inputs):
    t = float(np.asarray(inputs["t"]))
    ts = np.asarray(inputs["timestamps"], np.float32)
    bvv = np.asarray(inputs["boundary_values"], np.float32)  # [T,1,2000,64]
    i = int(np.searchsorted(ts, t))
    il, ir = max(i - 1, 0), min(i, ts.shape[0] - 1)
    if ts[il] == ts[ir]:
        bv_t = bvv[il][0]
    else:
        bv_t = (bvv[il] + (t - ts[il]) * (bvv[ir] - bvv[il]) / (ts[ir] - ts[il]))[0]
    bv_t = np.asarray(bv_t, np.float32)                      # [2000, 64]

    x0 = np.asarray(inputs["x_int"][0], np.float32)          # [50000, 64]
    ei = np.asarray(inputs["edge_index_int"])
    eb = np.asarray(inputs["edge_index_bound"])
    src = np.concatenate([ei[0], eb[0]]).astype(np.int64)
    dst = np.concatenate([ei[1], eb[1]]).astype(np.int64)
    E = src.shape[0]

    cnt = np.bincount(dst, minlength=N_INT).astype(np.float32)
    assert (cnt > 0).all(), "zero in-degree node: fused-weight path invalid"
    inv = 1.0 / cnt                                          # [N_INT]

    # table mapping for sources
    interior = src < N_INT
    c_src = src // NLOC
    l_src = src - c_src * NLOC
    grp = np.where(interior, np.where(l_src < A_LOC, 0, 1), 2).astype(np.int64)
    tix = np.where(
        interior,
        np.where(l_src < A_LOC, c_src * A_LOC + l_src,
                 c_src * B_LOC + (l_src - A_LOC)),
        src - N_INT,
    ).astype(np.int64)
    assert tix.max() < 32768

    core_of = dst // NLOC
    lloc = dst - core_of * NLOC
    tile_of = lloc // 128
    pos_of = lloc % 128

    key = (core_of * TILES + tile_of) * 3 + grp
    nkey = NCORE * TILES * 3
    counts = np.bincount(key, minlength=nkey).reshape(NCORE, TILES, 3)
    K = np.ceil(counts / 128).astype(np.int64).max(axis=0)   # [TILES, 3]
    chunk_meta, chunk_of, gather_insts, sgs = _chunk_layout(K)
    TOTCH = len(chunk_meta)
    SLOTS = TOTCH * 128

    # slot index for each edge
    order = np.argsort(key, kind="stable")
    run_start = np.zeros(nkey, np.int64)
    run_start[1:] = np.cumsum(np.bincount(key, minlength=nkey))[:-1]
    rank = np.arange(E, dtype=np.int64) - run_start[key[order]]
    # chunk base (in slots) for each (t, g)
    cb = np.zeros((TILES, 3), np.int64)
    for (tt, gg), (c0, kk) in chunk_of.items():
        cb[tt, gg] = c0 * 128
    slot = cb[tile_of[order], grp[order]] + rank

    idx_all = np.zeros((NCORE, SLOTS), np.int16)
    dstp_all = np.full((NCORE, SLOTS), 255.0, np.float32)
    invd_all = np.zeros((NCORE, SLOTS), np.float32)
    co = core_of[order]
    idx_all[co, slot] = tix[order].astype(np.int16)
    dstp_all[co, slot] = pos_of[order].astype(np.float32)
    invd_all[co, slot] = inv[dst[order]]

    # device layouts
    idx_lay = idx_all.reshape(NCORE, SLOTS // 16, 16).transpose(0, 2, 1)  # [8,16,W]
    idx_lay = np.ascontiguousarray(np.tile(idx_lay, (1, 8, 1)))           # [8,128,W]
    # prebuilt scatter one-hots: ohm[p, c*128+v] = (dst(slot c*128+p)==v)*inv
    ar = np.arange(128, dtype=np.float32)
    ohm_lay = []
    for c in range(NCORE):
        dst_c = dstp_all[c].reshape(TOTCH, 128).T          # [128, TOTCH]
        inv_c = invd_all[c].reshape(TOTCH, 128).T.astype(ml_dtypes.bfloat16).astype(np.float32)
        eq = dst_c[:, :, None] == ar[None, None, :]        # [128, TOTCH, 128]
        ohm = np.where(eq, inv_c[:, :, None], 0.0).astype(ml_dtypes.bfloat16)
        ohm_lay.append(np.ascontiguousarray(ohm.reshape(128, TOTCH * 128)))

    # layer-1 gather tables (bf16, rows padded to 128 channels)
    xpad = np.zeros((NCORE * NLOC, D_IN), np.float32)
    xpad[:N_INT] = x0
    t1a = np.zeros((NA, 128), ml_dtypes.bfloat16)
    t1b = np.zeros((NB, 128), ml_dtypes.bfloat16)
    t1c = np.zeros((NC_, 128), ml_dtypes.bfloat16)
    for c in range(NCORE):
        s = c * NLOC
        t1a[c * A_LOC:(c + 1) * A_LOC, :D_IN] = xpad[s:s + A_LOC]
        t1b[c * B_LOC:(c + 1) * B_LOC, :D_IN] = xpad[s + A_LOC:s + NLOC]
    t1c[:N_BOUND, :D_IN] = bv_t

    xT0 = np.zeros((NCORE, D_IN, NLOC), np.float32)
    for c in range(NCORE):
        xT0[c] = xpad[c * NLOC:(c + 1) * NLOC].T
    bvT0 = np.zeros((D_IN, NC_), np.float32)
    bvT0[:, :N_BOUND] = bv_t.T

    weights = {}
    for li in range(1, 5):
        d = DINS[li - 1]
        Wm = np.asarray(inputs[f"Wm{li}"], np.float64)
        bm = np.asarray(inputs[f"bm{li}"], np.float64)
        Ws = np.asarray(inputs[f"Ws{li}"], np.float64)
        bs = np.asarray(inputs[f"bs{li}"], np.float64)
        Wu = np.asarray(inputs[f"Wu{li}"], np.float64)
        bu = np.asarray(inputs[f"bu{li}"], np.float64)
        Wl, Wr = Wm[:, :d], Wm[:, d:]
        weights[f"waT{li}"] = np.ascontiguousarray((Wu @ Wl).T.astype(np.float32))
        weights[f"wbT{li}"] = np.ascontiguousarray((Wu @ Wr + Ws).T.astype(np.float32))
        weights[f"bc{li}"] = (Wu @ bm + bs + bu).astype(np.float32)[:, None]
        if li < 4:
            weights[f"wsT{li}"] = np.ascontiguousarray(Ws.T.astype(np.float32))
            weights[f"bs{li}"] = bs.astype(np.float32)[:, None]
    weights["bc4row"] = np.ascontiguousarray(weights["bc4"].T)  # [1, 64]

    shared = dict(t1a=np.asarray(t1a), t1b=np.asarray(t1b), t1c=np.asarray(t1c),
                  bvT0=bvT0, **weights)
    per_core = []
    for c in range(NCORE):
        per_core.append(dict(shared, idx=idx_lay[c], ohm=ohm_lay[c], xT0=xT0[c]))
    struct = dict(K=tuple(map(tuple, K)), TOTCH=TOTCH, SLOTS=SLOTS)
    return per_core, struct


# --------------------------------------------------------------------------
# device program
# --------------------------------------------------------------------------
def _build_nc(struct):
    K = [list(r) for r in struct["K"]]
    TOTCH = struct["TOTCH"]
    chunk_meta, chunk_of, gather_insts, sgs = _chunk_layout(K)
    W = TOTCH * 8  # idx columns

    nc = bacc.Bacc("TRN2", target_bir_lowering=False, debug=False,
                   num_devices=NCORE, num_swdge_queues=4)

    t1 = [nc.dram_tensor(n, [r, 128], BF16, kind="ExternalInput")
          for n, r in (("t1a", NA), ("t1b", NB), ("t1c", NC_))]
    idx_d = nc.dram_tensor("idx", [128, W], I16, kind="ExternalInput")
    ohm_d = nc.dram_tensor("ohm", [128, TOTCH * 128], BF16, kind="ExternalInput")
    xT0_d = nc.dram_tensor("xT0", [D_IN, NLOC], F32, kind="ExternalInput")
    bvT0_d = nc.dram_tensor("bvT0", [D_IN, NC_], F32, kind="ExternalInput")
    wd = {}
    for li in range(1, 5):
        d, do = DINS[li - 1], DOUTS[li - 1]
        wd[f"waT{li}"] = nc.dram_tensor(f"waT{li}", [d, do], F32, kind="ExternalInput")
        wd[f"wbT{li}"] = nc.dram_tensor(f"wbT{li}", [d, do], F32, kind="ExternalInput")
        wd[f"bc{li}"] = nc.dram_tensor(f"bc{li}", [do, 1], F32, kind="ExternalInput")
        if li < 4:
            wd[f"wsT{li}"] = nc.dram_tensor(f"wsT{li}", [d, 128], F32, kind="ExternalInput")
            wd[f"bs{li}"] = nc.dram_tensor(f"bs{li}", [128, 1], F32, kind="ExternalInput")
    wd["bc4row"] = nc.dram_tensor("bc4row", [1, 64], F32, kind="ExternalInput")
    out_d = nc.dram_tensor("out", [NLOC, DOUTS[3]], F32, kind="ExternalOutput")

    with tile.TileContext(nc) as tc:
        with (
            tc.tile_pool(name="const", bufs=1) as const,
            tc.tile_pool(name="xb", bufs=1) as xb,
            tc.tile_pool(name="gst", bufs=7) as gpool,
            tc.tile_pool(name="ohl", bufs=7) as ohlp,
            tc.tile_pool(name="stp", bufs=4) as stp,
            tc.tile_pool(name="misc", bufs=3) as misc,
            tc.tile_pool(name="tsg", bufs=2) as tsg,
            tc.tile_pool(name="pacc", bufs=5, space="PSUM") as pacc_p,
            tc.tile_pool(name="pup", bufs=2, space="PSUM") as pup,
            tc.tile_pool(name="ptr", bufs=1, space="PSUM") as ptrp,
            tc.tile_pool(name="dram", bufs=1, space="DRAM") as dram,
        ):
            # ---- resident constants
            idx_sb = const.tile([128, W], I16)
            nc.sync.dma_start(idx_sb[:], idx_d[:])
            idbf = const.tile([128, 128], BF16)
            make_identity(nc, idbf[:])
            idf32 = const.tile([64, 64], F32)
            make_identity(nc, idf32[:])
            ones_sb = const.tile([1, 128], F32)
            nc.vector.memset(ones_sb[:], 1.0)
            wsb = {}
            for name, hd in wd.items():
                w = const.tile(list(hd.shape), F32, name=f"w_{name}")
                nc.sync.dma_start(w[:], hd[:])
                wsb[name] = w

            # ---- x / bv feature buffers (channel-major fp32)
            x_sb = [xb.tile([D_IN, NLOC], F32, name="x0buf"),
                    xb.tile([128, NLOC], F32, name="xAbuf"),
                    xb.tile([128, NLOC], F32, name="xBbuf")]
            nc.sync.dma_start(x_sb[0][:], xT0_d[:])
            bv_sb = [xb.tile([D_IN, NC_], F32, name="bv0buf"),
                     xb.tile([128, NC_], F32, name="bvAbuf"),
                     xb.tile([128, NC_], F32, name="bvBbuf")]
            nc.sync.dma_start(bv_sb[0][:], bvT0_d[:])

            # ---- per-boundary DRAM tables
            tabs = {1: [t1[0][:], t1[1][:], t1[2][:]]}
            cc_in = {}
            for li in (2, 3, 4):
                ta = dram.tile([NA, 128], BF16, name=f"TA{li}", addr_space="Shared")
                tb = dram.tile([NB, 128], BF16, name=f"TB{li}", addr_space="Shared")
                tc_ = dram.tile([NC_, 128], BF16, name=f"TC{li}")
                tabs[li] = [ta, tb, tc_]
                cc_in[li] = [dram.tile([A_LOC, 128], BF16, name=f"ccA{li}"),
                             dram.tile([B_LOC, 128], BF16, name=f"ccB{li}")]

            for li in range(1, 5):
                d, do = DINS[li - 1], DOUTS[li - 1]
                xT = x_sb[0] if li == 1 else x_sb[1 + (li % 2)]
                xTn = x_sb[1 + ((li + 1) % 2)]      # layers 1..3 write here
                bvT = bv_sb[0] if li == 1 else bv_sb[1 + (li % 2)]
                bvTn = bv_sb[1 + ((li + 1) % 2)]
                tabA, tabB, tabC = tabs[li]
                tabsrc = [tabA, tabB, tabC]

                # gather instructions for this layer, keyed by chunk range
                gtiles = []   # (c0, n, gather tile, one-hot tile)
                def flush_sg(sg_insts):
                    for (g, c0, n) in sg_insts:
                        gt = gpool.tile([128, INST_CHUNKS, 128], BF16,
                                        name=f"g{li}", tag="gst")
                        nc.gpsimd.dma_gather(
                            out_ap=gt[:, :n, :],
                            in_ap=tabsrc[g][:, :],
                            idxs_ap=idx_sb[:, c0 * 8:(c0 + n) * 8],
                            num_idxs=n * 128,
                            num_idxs_reg=n * 128,
                            elem_size=128,
                            single_packet=False,
                            queue_num=len(gtiles) % 4,
                        )
                        ohl = ohlp.tile([128, INST_CHUNKS * 128], BF16,
                                        name=f"ohl{li}", tag="ohl")
                        nc.sync.dma_start(ohl[:, :n * 128],
                                          ohm_d[:, c0 * 128:(c0 + n) * 128])
                        gtiles.append((c0, n, gt, ohl))

                def g_slice(cg):
                    for (c0, n, gt, ohl) in reversed(gtiles):
                        if c0 <= cg < c0 + n:
                            return (gt[:, cg - c0, :d],
                                    ohl[:, (cg - c0) * 128:(cg - c0 + 1) * 128])
                    raise KeyError(cg)

                if li < 4:
                    # boundary-node update (replicated on every core); done at
                    # layer start so the C table write + bv' compute overlap
                    # the first supergroups' gathers.
                    for s in range(4):
                        pb = pup.tile([128, 512], F32, name=f"pb{li}", tag="p3")
                        nc.tensor.matmul(pb[:], lhsT=wsb[f"wsT{li}"][:],
                                         rhs=bvT[:d, s * 512:(s + 1) * 512],
                                         start=True, stop=True)
                        bex = misc.tile([128, 512], F32, name=f"bex{li}", tag="bex")
                        nc.scalar.activation(
                            out=bex[:], in_=pb[:],
                            func=mybir.ActivationFunctionType.Exp,
                            bias=wsb[f"bs{li}"][:])
                        if li < 3:
                            nc.scalar.activation(
                                out=bvTn[:, s * 512:(s + 1) * 512], in_=bex[:],
                                func=mybir.ActivationFunctionType.Ln, bias=1.0)
                        bvbf = misc.tile([128, 512], BF16, name=f"bvbf{li}", tag="bvbf")
                        nc.scalar.activation(
                            out=bvbf[:], in_=bex[:],
                            func=mybir.ActivationFunctionType.Ln, bias=1.0)
                        tstage = tsg.tile([128, SG, 128], BF16, name=f"tsb{li}", tag="ts")
                        for k in range(4):
                            tp = ptrp.tile([128, 128], BF16, name=f"tpb{li}", tag="tp")
                            nc.tensor.transpose(tp[:], bvbf[:, k * 128:(k + 1) * 128],
                                                idbf[:])
                            nc.vector.tensor_copy(out=tstage[:, k, :], in_=tp[:])
                        ov = tabs[li + 1][2][s * 512:(s + 1) * 512, :].rearrange(
                            "(k p) c -> p k c", p=128)
                        nc.sync.dma_start(out=ov, in_=tstage[:, :, :])

                inst_i = 0
                for sgi, sg in enumerate(sgs):
                    # issue this supergroup's gathers
                    mine = []
                    while inst_i < len(gather_insts):
                        g, c0, n = gather_insts[inst_i]
                        t0 = chunk_meta[c0][0]
                        if t0 in sg:
                            mine.append((g, c0, n))
                            inst_i += 1
                        else:
                            break
                    flush_sg(mine)
                    if li < 4 and sgi == 9:
                        # A half was fully staged ~2 supergroups ago (tile 31,
                        # sgi 7); all-gather it now -- late enough that the
                        # Pool engine won't stall waiting on the staging DMAs,
                        # early enough to overlap the remaining supergroups.
                        nc.gpsimd.collective_compute(
                            "AllGather", mybir.AluOpType.bypass,
                            replica_groups=[list(range(NCORE))],
                            ins=[cc_in[li + 1][0][:]],
                            outs=[tabs[li + 1][0][:]])

                    nsg = len(sg)
                    tstage = tsg.tile([128, SG, 128], BF16, name=f"ts{li}", tag="ts") \
                        if li < 4 else None
                    ostage = tsg.tile([128, SG, 64], F32, name=f"os{li}", tag="os") \
                        if li == 4 else None
                    for k, t in enumerate(sg):
                        chunks = []
                        for g in (0, 2, 1):   # consume in gather-arrival order
                            c0, kk = chunk_of[(t, g)]
                            chunks += list(range(c0, c0 + kk))
                        pacc = pacc_p.tile([d, 128], F32, name=f"pa{li}", tag="pacc")
                        for ci, cg in enumerate(chunks):
                            glhs, goh = g_slice(cg)
                            nc.tensor.matmul(
                                out=pacc[:], lhsT=glhs, rhs=goh,
                                start=(ci == 0), stop=(ci == len(chunks) - 1),
                            )
                        stile = stp.tile([d, 128], F32, name=f"st{li}", tag="st")
                        nc.vector.tensor_copy(out=stile[:], in_=pacc[:])
                        p3 = pup.tile([do, 128], F32, name=f"p3{li}", tag="p3")
                        nc.tensor.matmul(p3[:], lhsT=wsb[f"waT{li}"][:], rhs=stile[:],
                                         start=True, stop=False)
                        nc.tensor.matmul(p3[:], lhsT=wsb[f"wbT{li}"][:],
                                         rhs=xT[:d, t * 128:(t + 1) * 128],
                                         start=False, stop=(li == 4 and False) or li < 4)
                        if li < 4:
                            # softplus(z) = ln(1 + exp(z)); Softplus itself is
                            # not in any loadable ACT table, Exp+Ln are.
                            ex = misc.tile([128, 128], F32, name=f"ex{li}", tag="ex")
                            nc.scalar.activation(
                                out=ex[:], in_=p3[:],
                                func=mybir.ActivationFunctionType.Exp,
                                bias=wsb[f"bc{li}"][:])
                            nc.scalar.activation(
                                out=xTn[:, t * 128:(t + 1) * 128], in_=ex[:],
                                func=mybir.ActivationFunctionType.Ln, bias=1.0)
                            xbf = misc.tile([128, 128], BF16, name=f"xbf{li}", tag="xbf")
                            nc.scalar.activation(
                                out=xbf[:], in_=ex[:],
                                func=mybir.ActivationFunctionType.Ln, bias=1.0)
                            tp = ptrp.tile([128, 128], BF16, name=f"tp{li}", tag="tp")
                            nc.tensor.transpose(tp[:], xbf[:], idbf[:])
                            nc.vector.tensor_copy(out=tstage[:, k, :], in_=tp[:])
                        else:
                            nc.tensor.matmul(p3[:], lhsT=wsb["bc4row"][:],
                                             rhs=ones_sb[:], start=False, stop=True)
                            ob = misc.tile([64, 128], F32, name="ob4", tag="xbf")
                            nc.scalar.activation(
                                out=ob[:], in_=p3[:],
                                func=mybir.ActivationFunctionType.Copy)
                            tp4 = ptrp.tile([128, 64], F32, name="tp4", tag="tp")
                            nc.tensor.transpose(tp4[:], ob[:], idf32[:])
                            nc.vector.tensor_copy(out=ostage[:, k, :], in_=tp4[:])

                    # flush staging to DRAM
                    r0 = sg[0] * 128
                    nrow = nsg * 128
                    if li < 4:
                        dst_tab = cc_in[li + 1][0] if sg[0] < 32 else cc_in[li + 1][1]
                        roff = r0 if sg[0] < 32 else r0 - A_LOC
                        ov = dst_tab[roff:roff + nrow, :].rearrange(
                            "(k p) c -> p k c", p=128)
                        nc.sync.dma_start(out=ov, in_=tstage[:, :nsg, :])
                    else:
                        ov = out_d[r0:r0 + nrow, :].rearrange("(k p) c -> p k c", p=128)
                        nc.sync.dma_start(out=ov, in_=ostage[:, :nsg, :])

                if li < 4:
                    # all-gather the (smaller) B half at layer end
                    nc.gpsimd.collective_compute(
                        "AllGather", mybir.AluOpType.bypass,
                        replica_groups=[list(range(NCORE))],
                        ins=[cc_in[li + 1][1][:]], outs=[tabs[li + 1][1][:]])

    nc.compile()
    return nc


# --------------------------------------------------------------------------
TRACE = False          # test harness can flip this to capture an NTFF profile
TRACE_CORES = [0]      # which cores to profile (fewer = faster test turnaround)
last_results = None    # BassKernelResults of the most recent kernel() call


def kernel(**inputs) -> np.ndarray:
    global last_results
    per_core, struct = _host_prep(inputs)
    key = struct["K"]
    if key not in _cache:
        _cache[key] = _build_nc(struct)
    nc = _cache[key]
    kw = {}
    if TRACE:
        kw = dict(trace=True, trace_cores=list(TRACE_CORES))
    res = run_bass_kernel_spmd(nc, per_core, core_ids=list(range(NCORE)), **kw)
    last_results = res
    out = np.concatenate([res.results[c]["out"] for c in range(NCORE)], axis=0)
    return out[None, :N_INT, :].astype(np.float32)

